# revision 12
# baseline (speedup 1.0000x reference)
"""Multi-head attention with RoPE on 8 Trainium2 NeuronCores.

Sharding: core c handles batch b = c//2 and head-group hg = c%2 (8 of 16
heads).  Data-parallel over batch, tensor-parallel over heads; the
row-parallel wo all-reduce (2 cores per batch) happens on the host during
the gather/unshard step.

v2 design (bf16 on-chip, f32 PSUM accumulation):
  * Pipelined rounds over s-chunks of 512: round n does K/Q/V projection
    chunk n (PE-dense) while attention for chunk n-1 runs (ACT-bound exp).
    The Tile list-scheduler interleaves them, keeping the PE warm (HAM) and
    the ScalarE exp stream saturated.
  * Scores for the 2 heads of a KT/QT tile are row-packed: head A uses PE
    array rows 0-63, head B rows 64-127 (tile_position auto-derived from
    base partitions), writing separate PSUM bank-pairs -> ~2x scores rate.
  * One exp ACTIVATE per (head-pair, k-block): [128, 1024] spanning 2 PSUM
    banks, amortizing the ~350-cycle ACT instruction overhead.
  * V is stored [s, 8*(64+1)] with a ones column per head so the attn@V
    matmul emits the softmax denominator as row 64 for free.
  * Normalization: denominator row -> SBUF (DVE), K=1 ones-outer matmul
    broadcasts it to 64 partitions, reciprocal_approx_fast (single DVE op),
    then one multiply writes the normalized A tile.
  * wo partials are computed per chunk and DMA'd out in bf16; the host sums
    the two head-group partials (the "all-reduce") and transposes.
"""

import sys
import types

sys.path.insert(0, "/opt/trn_rl_repo")

import numpy as np
import ml_dtypes

import concourse.bacc as bacc
import concourse.mybir as mybir
import concourse.tile as tile
from concourse.bass_utils import run_bass_kernel_spmd

# Problem constants (hardcoded per contract)
B, S, D = 4, 2048, 1024
H = 16
DH = D // H          # 64
THETA = 10000.0
NCORES = 8
HG = 2               # head groups (tensor-parallel factor)
HD = D // HG         # 512 = per-core heads dim
NH = H // HG         # 8 heads per core
P = 128
SC = 512             # s-chunk
NSC = S // SC        # 4
NKB = S // P         # 16 k-blocks
NDB = D // P         # 8 d-blocks (contraction for projections)
SCALE = 1.0 / np.sqrt(np.float32(DH))

F32 = mybir.dt.float32
BF16 = mybir.dt.bfloat16

BF = ml_dtypes.bfloat16


def _install_ntff_hook():
    """Best-effort: register the axon NTFF profile hook so trace=True works."""
    try:
        import antenv

        if "antenv.axon_hooks" in sys.modules:
            return
        sys.path.insert(0, "/root/.axon_site/trn_agent_boot")
        import trn_boot

        hook = trn_boot._ntff_profile_via_ctypes("/opt/axon/libaxon_pjrt.so")
        mod = types.ModuleType("antenv.axon_hooks")
        mod.get_axon_ntff_profile_hook = lambda: hook
        mod.set_axon_ntff_profile_hook = lambda h: None
        sys.modules["antenv.axon_hooks"] = mod
        antenv.axon_hooks = mod
    except Exception:
        pass


def build_program():
    nc = bacc.Bacc("TRN2", target_bir_lowering=False, debug=False,
                   num_devices=NCORES)

    xt_d = nc.dram_tensor("xt", [D, S], BF16, kind="ExternalInput")
    wqt_d = nc.dram_tensor("wqt", [D, HD], BF16, kind="ExternalInput")
    wkt_d = nc.dram_tensor("wkt", [D, HD], BF16, kind="ExternalInput")
    wvt_d = nc.dram_tensor("wvt", [D, HD], BF16, kind="ExternalInput")
    wot_d = nc.dram_tensor("wot", [HD, D], BF16, kind="ExternalInput")
    cf_d = nc.dram_tensor("cfull", [P, S], BF16, kind="ExternalInput")
    sf_d = nc.dram_tensor("sfull", [P, S], BF16, kind="ExternalInput")
    tri_d = nc.dram_tensor("tri", [P, P], BF16, kind="ExternalInput")
    ones_d = nc.dram_tensor("ones", [P, NH], BF16, kind="ExternalInput")
    out_d = nc.dram_tensor("outT", [D, S], BF16, kind="ExternalOutput")

    EXP = mybir.ActivationFunctionType.Exp
    MULT = mybir.AluOpType.mult
    ADD = mybir.AluOpType.add

    with tile.TileContext(nc) as tc:
        with (
            tc.tile_pool(name="xp", bufs=1) as xp,        # x^T persistent
            tc.tile_pool(name="qk", bufs=1) as qk,        # QT/KT persistent
            tc.tile_pool(name="vgp", bufs=1) as vgp,      # V_aug persistent
            tc.tile_pool(name="ap", bufs=1) as app,       # A persistent
            tc.tile_pool(name="wp", bufs=1) as wp,        # weights persistent
            tc.tile_pool(name="misc", bufs=1) as misc,    # cf/sf/tri/ones
            tc.tile_pool(name="atp", bufs=4) as atp,      # exp outputs
            tc.tile_pool(name="swp", bufs=4) as swp,      # rope swap staging
            tc.tile_pool(name="otp", bufs=4) as otp,      # wo out staging
            tc.tile_pool(name="rqp", bufs=4) as rqp,      # denom rows
            tc.tile_pool(name="rbp", bufs=4) as rbp,      # recip bcast
            tc.tile_pool(name="psS", bufs=2, space="PSUM") as psS,  # scores
            tc.tile_pool(name="psV", bufs=2, space="PSUM") as psV,  # attn@V
            tc.tile_pool(name="psA", bufs=2, space="PSUM") as psA,  # proj/wo/bcast
        ):
            with nc.allow_low_precision(reason="bf16 kernel by design"):
                _body(nc, tc, xp, qk, vgp, app, wp, misc, atp, swp, otp,
                      rqp, rbp, psS, psV, psA,
                      xt_d, wqt_d, wkt_d, wvt_d, wot_d, cf_d, sf_d, tri_d,
                      ones_d, out_d, EXP, MULT, ADD)

    nc.compile()
    return nc


def _body(nc, tc, xp, qk, vgp, app, wp, misc, atp, swp, otp, rqp, rbp,
          psS, psV, psA,
          xt_d, wqt_d, wkt_d, wvt_d, wot_d, cf_d, sf_d, tri_d, ones_d,
          out_d, EXP, MULT, ADD):
    # ---- persistent SBUF tiles ----
    xt = []
    for k in range(NDB):
        t = xp.tile([P, S], BF16, tag=f"x{k}", name=f"xt{k}")
        nc.sync.dma_start(t[:], xt_d[P * k:P * (k + 1), :])
        xt.append(t)

    def load_w(dram, pfx, n, width):
        ts_ = []
        for k in range(n):
            t = wp.tile([P, width], BF16, tag=f"{pfx}{k}", name=f"{pfx}{k}")
            nc.scalar.dma_start(t[:], dram[P * k:P * (k + 1), :])
            ts_.append(t)
        return ts_

    wq_t = load_w(wqt_d, "wq", NDB, HD)
    wk_t = load_w(wkt_d, "wk", NDB, HD)
    wv_t = load_w(wvt_d, "wv", NDB, HD)
    wo_t = load_w(wot_d, "wo", HD // P, D)

    cf = misc.tile([P, S], BF16, tag="cf", name="cf")
    nc.sync.dma_start(cf[:], cf_d[:])
    sf = misc.tile([P, S], BF16, tag="sf", name="sf")
    nc.sync.dma_start(sf[:], sf_d[:])
    tri = misc.tile([P, P], BF16, tag="tri", name="tri")
    nc.sync.dma_start(tri[:], tri_d[:])
    ones8 = misc.tile([P, NH], BF16, tag="ones8", name="ones8")
    nc.sync.dma_start(ones8[:], ones_d[:])

    QT = [qk.tile([P, S], BF16, tag=f"qt{m}", name=f"qt{m}")
          for m in range(HD // P)]
    KT = [qk.tile([P, S], BF16, tag=f"kt{m}", name=f"kt{m}")
          for m in range(HD // P)]
    Vg = [vgp.tile([P, NH * (DH + 1)], BF16, tag=f"vg{j}", name=f"vg{j}")
          for j in range(NKB)]
    A = [app.tile([P, S], BF16, tag=f"a{m}", name=f"a{m}")
         for m in range(HD // P)]

    # ---- building blocks ----
    def proj_chunk(w_tiles, out_tiles, n):
        """out[m][:, n*SC:+SC] = sum_k w[k][:, m*128:+128].T @ xt[k][:, nSC:+SC]"""
        for m in range(HD // P):
            ps = psA.tile([P, SC], F32, tag="pa", name="pps")
            for k in range(NDB):
                nc.tensor.matmul(
                    ps[:],
                    (w_tiles[k][:, P * m:P * (m + 1)]),
                    (xt[k][:, SC * n:SC * (n + 1)]),
                    start=(k == 0), stop=(k == NDB - 1),
                )
            nc.scalar.copy(out_tiles[m][:, SC * n:SC * (n + 1)], ps[:])

    def rope_chunk(t, n, dma_eng=None):
        """In-place RoPE on t[:, n*SC:+SC].
        rows of t: [h0.x1 | h0.x2 | h1.x1 | h1.x2] (32 each)
        out = t * cf + swap32pairs(t) * sf
        """
        cs = slice(SC * n, SC * (n + 1))
        sw = swp.tile([P, SC], BF16, tag="sw", name="sw")
        for g in range(4):
            src = (g ^ 1) * 32
            eng = nc.sync if dma_eng is None else dma_eng
            eng.dma_start(sw[g * 32:(g + 1) * 32, :], t[src:src + 32, cs])
        nc.gpsimd.tensor_tensor(sw[:], sw[:], sf[:, cs], MULT)
        nc.vector.tensor_tensor(t[:, cs], t[:, cs], cf[:, cs], MULT)
        nc.vector.tensor_tensor(t[:, cs], t[:, cs], sw[:], ADD)

    def vproj_block(j):
        """Vg[j] = [x_block @ wv^T | ones] per head: [128, 8*(64+1)]"""
        vt = Vg[j]
        v3 = vt[:].rearrange("p (h c) -> p h c", h=NH)
        ps = psA.tile([P, HD], F32, tag="pa", name="vps")
        for k in range(NDB):
            nc.tensor.matmul(
                ps[:],
                (xt[k][:, P * j:P * (j + 1)]),
                (wv_t[k][:]),
                start=(k == 0), stop=(k == NDB - 1),
            )
        nc.vector.tensor_copy(v3[:, :, 0:DH],
                              ps[:].rearrange("p (h c) -> p h c", h=NH))
        nc.vector.tensor_copy(v3[:, :, DH:DH + 1], ones8[:, :, None])

    def attention_chunk(qc):
        """Attention for q-chunk qc over all 4 head-pair tiles."""
        qs = slice(SC * qc, SC * (qc + 1))
        nj = 4 * qc + 4
        for ht in range(HD // P):
            av = [psV.tile([DH + 1, SC], F32, tag="av", name=f"av{hh}")
                  for hh in range(2)]
            for j in range(nj):
                d = j - 4 * qc
                q0 = P * d if d >= 0 else 0
                ps = psS.tile([P, 2 * SC], F32, tag="sc", name="sc")
                at = atp.tile([P, 2 * SC], BF16, tag="at", name="at")
                for hh in range(2):
                    nc.tensor.matmul(
                        ps[:, SC * hh + q0:SC * (hh + 1)],
                        (KT[ht][DH * hh:DH * (hh + 1), P * j:P * (j + 1)]),
                        (QT[ht][DH * hh:DH * (hh + 1),
                                SC * qc + q0:SC * (qc + 1)]),
                        start=True, stop=True,
                    )
                if q0 == 0:
                    # one exp for both heads (2 PSUM banks, 1024 wide)
                    nc.scalar.activation(at[:], ps[:], EXP,
                                         scale=float(SCALE))
                else:
                    # one strided exp over both heads' valid [q0:SC) ranges
                    ps3 = ps[:].rearrange("p (h q) -> p h q", h=2)
                    at3 = at[:].rearrange("p (h q) -> p h q", h=2)
                    nc.scalar.activation(at3[:, :, q0:SC], ps3[:, :, q0:SC],
                                         EXP, scale=float(SCALE))
                if d >= 0:
                    q1 = P * d
                    for hh in range(2):
                        nc.vector.tensor_tensor(
                            at[:, SC * hh + q1:SC * hh + q1 + P],
                            at[:, SC * hh + q1:SC * hh + q1 + P],
                            tri[:], MULT)
                qa = P * d if d >= 0 else 0
                for hh in range(2):
                    h = 2 * ht + hh
                    nc.tensor.matmul(
                        av[hh][:, qa:SC],
                        (Vg[j][:, (DH + 1) * h:(DH + 1) * (h + 1)]),
                        (at[:, SC * hh + qa:SC * (hh + 1)]),
                        start=(j == 0), stop=(j == nj - 1),
                    )
            # normalize: A[ht][64*hh:+64, qs] = av[0:64] / av[64]
            for hh in range(2):
                rq = rqp.tile([1, SC], BF16, tag="rq", name="rq")
                nc.vector.tensor_copy(rq[:], av[hh][DH:DH + 1, :])
                bc = psA.tile([DH, SC], F32, tag="pa", name="bc")
                nc.tensor.matmul(bc[:], (tri[0:1, 0:DH]), (rq[:]),
                                 start=True, stop=True)
                rb = rbp.tile([DH, SC], F32, tag="rb", name="rb")
                nc.vector.reciprocal_approx_fast(rb[:], bc[:])
                nc.vector.tensor_tensor(
                    A[ht][DH * hh:DH * (hh + 1), qs],
                    av[hh][0:DH, :], rb[:], MULT)

    def wo_chunk(qc):
        """outT[:, qc*SC:+SC] partial = wo_p @ A[:, chunk]"""
        for m in range(D // P):
            ps = psA.tile([P, SC], F32, tag="pa", name="wps")
            for k in range(HD // P):
                nc.tensor.matmul(
                    ps[:],
                    (wo_t[k][:, P * m:P * (m + 1)]),
                    (A[k][:, SC * qc:SC * (qc + 1)]),
                    start=(k == 0), stop=(k == HD // P - 1),
                )
            ot = otp.tile([P, SC], BF16, tag="ot", name="ot")
            nc.vector.tensor_copy(ot[:], ps[:])
            nc.gpsimd.dma_start(
                out_d[P * m:P * (m + 1), SC * qc:SC * (qc + 1)], ot[:])

    # ---- pipelined rounds ----
    # attention(n-1) is emitted BEFORE round-n projections: the Tile
    # list-scheduler pops ready work by priority (program order), so this
    # keeps the ScalarE exp stream saturated while projection matmuls fill
    # the Tensor engine's idle slots.
    for n in range(NSC):
        if n > 0:
            attention_chunk(n - 1)
            wo_chunk(n - 1)
        proj_chunk(wk_t, KT, n)
        for m in range(HD // P):
            rope_chunk(KT[m], n)
        proj_chunk(wq_t, QT, n)
        for m in range(HD // P):
            rope_chunk(QT[m], n, dma_eng=nc.gpsimd)
        for j in range(4 * n, 4 * n + 4):
            vproj_block(j)
    attention_chunk(NSC - 1)
    wo_chunk(NSC - 1)


_NC_CACHE = []


def _get_nc():
    if not _NC_CACHE:
        _NC_CACHE.append(build_program())
    return _NC_CACHE[0]


def _host_tables(token_positions):
    pos = np.asarray(token_positions).astype(np.float32)
    inv_freq = np.float32(THETA) ** (
        -np.arange(0, DH, 2, dtype=np.float32) / np.float32(DH))
    ang = pos[:, None] * inv_freq[None, :]            # [S, 32] f32
    cos_t = np.ascontiguousarray(np.cos(ang).T)        # [32, S]
    sin_t = np.ascontiguousarray(np.sin(ang).T)
    cfull = np.tile(cos_t, (4, 1)).astype(np.float32)  # [128, S]
    sfull = np.concatenate([-sin_t, sin_t, -sin_t, sin_t], 0).astype(np.float32)
    return cfull, sfull


def _in_maps(in_features, token_positions, wq, wk, wv, wo):
    x = np.asarray(in_features, dtype=np.float32)
    wq = np.asarray(wq, dtype=np.float32)
    wk = np.asarray(wk, dtype=np.float32)
    wv = np.asarray(wv, dtype=np.float32)
    wo = np.asarray(wo, dtype=np.float32)

    cfull, sfull = _host_tables(token_positions)
    tri = np.triu(np.ones((P, P), dtype=np.float32))   # keep k_row <= q_col
    ones = np.ones((P, NH), dtype=np.float32)

    # per-head row permutation: evens then odds
    perm1 = np.concatenate([np.arange(0, DH, 2), np.arange(1, DH, 2)])
    perm = np.concatenate([h * DH + perm1 for h in range(NH)])

    in_maps = []
    for c in range(NCORES):
        b, hg = divmod(c, HG)
        sl = slice(hg * HD, (hg + 1) * HD)
        in_maps.append({
            "xt": np.ascontiguousarray(x[b].T).astype(BF),
            "wqt": np.ascontiguousarray(wq[sl][perm].T).astype(BF),
            "wkt": np.ascontiguousarray(wk[sl][perm].T).astype(BF),
            "wvt": np.ascontiguousarray(wv[sl].T).astype(BF),
            "wot": np.ascontiguousarray(wo[:, sl].T).astype(BF),
            "cfull": cfull.astype(BF),
            "sfull": sfull.astype(BF),
            "tri": tri.astype(BF),
            "ones": ones.astype(BF),
        })
    return in_maps


def kernel(in_features, token_positions, wq, wk, wv, wo):
    _install_ntff_hook()
    in_maps = _in_maps(in_features, token_positions, wq, wk, wv, wo)
    nc = _get_nc()
    res = run_bass_kernel_spmd(nc, in_maps, list(range(NCORES)))

    out = np.empty((B, S, D), dtype=np.float32)
    for b in range(B):
        acc = (res.results[2 * b]["outT"].astype(np.float32)
               + res.results[2 * b + 1]["outT"].astype(np.float32))
        out[b] = acc.T
    return out
